# revision 63
# baseline (speedup 1.0000x reference)
"""Differential attention (B=2, S=2048, D=2048, H=16) on 8 Trainium2 cores.

Sharding: core c -> batch b=c//4, head group hg=c%4 (4 heads each).
Each core computes qkv projection for its head columns, RoPE, differential
attention, per-head LayerNorm, and a partial @W_o over its 512 vd rows.
Host sums the 4 partials per batch. No duplicated FLOPs, no collectives.

Single-pass structure (per 512-token chunk): token-major qkv projection,
RoPE along the free axis, PE transposes to d-major K/Q kept in SBUF,
attention for all heads of that chunk, per-head LN, partial @W_o with the
output streamed out per chunk. Softmax denominators accumulate on the DVE
(fp16, 2x mode) instead of burning PE passes; LN stats stay replicated
[128,512] so no broadcast DMAs; LN stats are deferred one iteration so the
PE never waits on the DVE/ACT epilogue chain.
"""
import sys

sys.path.insert(0, "/opt/trn_rl_repo")

import numpy as np

B, S, D = 2, 2048, 2048
H = 16
HD = D // (2 * H)          # 64 per-map head dim
DH = 2 * HD                # 128 per-head dim
HPC = H // 4               # 4 heads per core
NCORES = 8
SCALE = HD ** -0.5         # 0.125
NEG = -8.0e9               # mask add value pre-scale (-1e9 / SCALE)
OUT_MULT = 1.0 - 0.8       # (1 - LBDA_INIT)

# module-level knobs / results for test.py
TRACE = False
TRACE_DIR = None
LAST_RESULTS = None
LAST_EXEC_NS = None

_PROGRAM_CACHE = {}


def build_program(s=S, lbda=0.5):
    """Per-core Bass program (SPMD: same program on all 8 cores)."""
    import concourse.bass as bass
    import concourse.tile as tile
    from concourse import bacc, mybir
    from concourse.bass import ts, ds

    f32 = mybir.dt.float32
    f32r = mybir.dt.float32r
    fp16 = mybir.dt.float16
    AF = mybir.ActivationFunctionType
    OP = mybir.AluOpType

    NCH = s // 512              # token chunks of 512
    KT = s // 128               # k tiles of 128
    KO = D // 128               # contraction chunks over D

    nc = bacc.Bacc()
    xT = nc.declare_dram_parameter("xT", [D, s], fp16, isOutput=False)
    wqk = nc.declare_dram_parameter("wqk", [D, 8 * 128], fp16, isOutput=False)
    wv = nc.declare_dram_parameter("wv", [D, HPC * DH], fp16, isOutput=False)
    wo = nc.declare_dram_parameter("wo", [HPC * DH, D], fp16, isOutput=False)
    csT = nc.declare_dram_parameter("csT", [s, 256], fp16, isOutput=False)
    snT = nc.declare_dram_parameter("snT", [s, 256], fp16, isOutput=False)
    gb = nc.declare_dram_parameter("gb", [128, 2 * HPC], f32, isOutput=False)
    mw = nc.declare_dram_parameter("mw", [128, 256], f32, isOutput=False)
    idn = nc.declare_dram_parameter("idn", [128, 128], fp16, isOutput=False)
    out = nc.declare_dram_parameter("out", [s, D], fp16, isOutput=True)

    r = lambda ap: ap.bitcast(f32r)

    xT3 = xT.rearrange("(ko p) t -> p ko t", p=128)
    wqk3 = wqk.rearrange("(ko p) c -> p ko c", p=128)
    wv3 = wv.rearrange("(ko p) c -> p ko c", p=128)
    wo3 = wo.rearrange("(h d) c -> d h c", h=HPC)
    cs3 = csT.rearrange("(n p) f -> p n f", p=128)
    sn3 = snT.rearrange("(n p) f -> p n f", p=128)

    with tile.TileContext(nc) as tc, \
         nc.allow_low_precision(reason="fp16 intermediates; rel-err gate 2e-2"):
        with tc.tile_pool(name="pw", bufs=1) as pw, \
             tc.tile_pool(name="pkv", bufs=1) as pkv, \
             tc.tile_pool(name="px", bufs=2) as px, \
             tc.tile_pool(name="pcs", bufs=2) as pcs, \
             tc.tile_pool(name="pqt", bufs=2) as pqt, \
             tc.tile_pool(name="pat", bufs=2) as pat, \
             tc.tile_pool(name="pst", bufs=2) as pst, \
             tc.tile_pool(name="peo", bufs=2) as peo, \
             tc.tile_pool(name="prt", bufs=1) as prt, \
             tc.tile_pool(name="pe", bufs=6) as pe_pool, \
             tc.tile_pool(name="pes", bufs=2) as pes, \
             tc.tile_pool(name="pt", bufs=1) as pt:
            # ---- persistent weights / tables ------------------------------
            idn_sb = pw.tile([128, 128], fp16)
            wqk_sb = pw.tile([128, KO, 8 * 128], fp16)
            wv_sb = pw.tile([128, KO, HPC * DH], fp16)
            for kq in range(4):
                nc.gpsimd.dma_start(wqk_sb[:, ts(kq, 4), :], wqk3[:, ts(kq, 4), :])
                nc.gpsimd.dma_start(wv_sb[:, ts(kq, 4), :], wv3[:, ts(kq, 4), :])
            gb_sb = pw.tile([128, 2 * HPC], f32)
            nc.gpsimd.dma_start(gb_sb[:], gb[:])
            mw_sb = pw.tile([128, 256], f32)
            nc.gpsimd.dma_start(mw_sb[:], mw[:])
            ones16 = pw.tile([128, 128], fp16)
            nc.vector.memset(ones16[:], 1.0)
            ones128 = pw.tile([128, 128], fp16)
            nc.vector.memset(ones128[:], 1.0 / DH)
            eps_sb = pw.tile([128, 1], f32)
            nc.vector.memset(eps_sb[:], 1e-5)
            dummy_sb = pw.tile([128, 1], f32)

            # persistent K / V for the whole sequence (this core's 4 heads)
            kt_sb = pkv.tile([128, HPC, s], fp16)        # [d, h, tok]
            vt_sb = pkv.tile([128, KT, HPC * DH], fp16)  # [tok%128, ktile, (h,d)]

            xts, csts, snts = {}, {}, {}

            def load_x(c, split=False):
                t = px.tile([128, KO, 512], fp16, tag="xt", name=f"xt{c}")
                if split:
                    for kq in range(4):
                        nc.sync.dma_start(t[:, ts(kq, 4), :],
                                          xT3[:, ts(kq, 4), ts(c, 512)])
                else:
                    nc.sync.dma_start(t[:], xT3[:, :, ts(c, 512)])
                xts[c] = ("whole", t)

            def load_cs(c):
                csts[c] = pcs.tile([128, 4, 256], fp16, tag="cs", name=f"cs{c}")
                nc.sync.dma_start(csts[c][:], cs3[:, ts(c, 4), :])
                snts[c] = pcs.tile([128, 4, 256], fp16, tag="sn", name=f"sn{c}")
                nc.sync.dma_start(snts[c][:], sn3[:, ts(c, 4), :])

            qts = {}
            attn_sbs = {}

            stats_tail = [None]

            def emit_proj(c, pq, pk, pv, ptr, pso):
                """Token-major qkv projection + RoPE + transposes for chunk c."""
                kind, xt = xts.pop(c)

                def xko(ko, sub):
                    return xt[:, ko, ts(sub, 128)]
                cst, snt = csts[c], snts[c]
                qts[c] = pqt.tile([128, HPC, 512], fp16, tag="qt", name=f"qt{c}")
                qt = qts[c]
                for sub in range(4):
                    Pq = pq.tile([128, 512], f32, tag="pq", name=f"Pq{c}_{sub}")
                    Pk = pk.tile([128, 512], f32, tag="pk", name=f"Pk{c}_{sub}")
                    Pv = pv.tile([128, 512], f32, tag="pv", name=f"Pv{c}_{sub}")
                    for ko in range(KO):
                        nc.tensor.matmul(Pq[:], xko(ko, sub),
                                         wqk_sb[:, ko, 0:512],
                                         start=(ko == 0), stop=(ko == KO - 1))
                    for ko in range(KO):
                        nc.tensor.matmul(Pk[:], xko(ko, sub),
                                         wqk_sb[:, ko, 512:1024],
                                         start=(ko == 0), stop=(ko == KO - 1))
                    if sub == 1 and stats_tail[0] is not None:
                        stats_tail[0](pso)
                        stats_tail[0] = None
                    for ko in range(KO):
                        nc.tensor.matmul(Pv[:], xko(ko, sub),
                                         wv_sb[:, ko, :],
                                         start=(ko == 0), stop=(ko == KO - 1))
                    # RoPE along free axis; outputs written head-contiguous
                    # (h, m, eo, j) so each transpose input is one 128-col slice
                    cc = cst[:, sub, :].rearrange("p (h m j) -> p h m j",
                                                  h=HPC, m=2)
                    sn_ = snt[:, sub, :].rearrange("p (h m j) -> p h m j",
                                                   h=HPC, m=2)
                    qeo = peo.tile([128, 512], fp16, tag="qeo")
                    keo = peo.tile([128, 512], fp16, tag="keo")
                    for P, dst in ((Pq, qeo), (Pk, keo)):
                        E = P[:, 0:256].rearrange("p (h m j) -> p h m j",
                                                  h=HPC, m=2)
                        O = P[:, 256:512].rearrange("p (h m j) -> p h m j",
                                                    h=HPC, m=2)
                        d4 = dst.rearrange("p (h m eo j) -> p h m eo j",
                                           h=HPC, m=2, eo=2)
                        t1 = prt.tile([128, 4, 2, 32], fp16, tag="rt1")
                        t2 = prt.tile([128, 4, 2, 32], fp16, tag="rt2")
                        nc.vector.tensor_tensor(t1[:], E, cc, OP.mult)
                        nc.vector.tensor_tensor(t2[:], O, sn_, OP.mult)
                        nc.vector.tensor_tensor(d4[:, :, :, 0, :], t1[:], t2[:],
                                                OP.subtract)
                        t3 = prt.tile([128, 4, 2, 32], fp16, tag="rt1")
                        t4 = prt.tile([128, 4, 2, 32], fp16, tag="rt2")
                        nc.vector.tensor_tensor(t3[:], E, sn_, OP.mult)
                        nc.vector.tensor_tensor(t4[:], O, cc, OP.mult)
                        nc.vector.tensor_tensor(d4[:, :, :, 1, :], t3[:], t4[:],
                                                OP.add)
                    # v -> vt_sb directly from PSUM
                    nc.scalar.copy(vt_sb[:, 4 * c + sub, :], Pv[:])
                    # transposes: [tok, d] -> [d, tok] per head
                    TQ = ptr.tile([128, 512], fp16, tag="tq")
                    TK = ptr.tile([128, 512], fp16, tag="tk")
                    for h in range(HPC):
                        nc.tensor.transpose(TQ[:, ts(h, 128)],
                                            qeo[:, ds(h * 128, 128)], idn_sb[:])
                        nc.tensor.transpose(TK[:, ts(h, 128)],
                                            keo[:, ds(h * 128, 128)], idn_sb[:])
                    nc.scalar.copy(
                        qt[:, :, ds(sub * 128, 128)],
                        TQ.rearrange("p (h t) -> p h t", h=HPC))
                    nc.scalar.copy(
                        kt_sb[:, :, ds(c * 512 + sub * 128, 128)],
                        TK.rearrange("p (h t) -> p h t", h=HPC))

            def emit_attn_iter(h, c, pap, mid=None, fillers=None, fstate=None, la=1, nofill=False):
                """One attention iteration (head h, query chunk c)."""
                qt = qts[c]
                klim = 4 * c + 4
                es12 = pes.tile([128, 2, 512], fp16, tag="es", name=f"es_{h}_{c}")
                mw2 = mw_sb.rearrange("p (m w) -> p m w", m=2)

                def c0_of(ki):
                    p = ki - 4 * c
                    if p <= 0:
                        return 0
                    return 128 * p if p < 3 else 256

                def emit_scores(ki):
                    c0 = c0_of(ki)
                    p = ki - 4 * c
                    s12 = pap.tile([128, 2, 512], f32, tag="s",
                                   name=f"s_{ki}_{h}_{c}")
                    nc.tensor.matmul(s12[:, 0, c0:], kt_sb[0:64, h, ts(ki, 128)],
                                     qt[0:64, h, c0:], start=True, stop=True)
                    nc.tensor.matmul(s12[:, 1, c0:], kt_sb[64:128, h, ts(ki, 128)],
                                     qt[64:128, h, c0:], start=True, stop=True)
                    if p >= 0:
                        if p < 3:
                            nc.vector.tensor_tensor(
                                s12[:, :, ds(128 * p, 128)],
                                s12[:, :, ds(128 * p, 128)],
                                mw2[:, :, 0:128], OP.add)
                        else:
                            # cols 256:384 are fully masked; apply only the
                            # triangular mask and skip their exp entirely
                            nc.vector.tensor_tensor(
                                s12[:, :, 384:512], s12[:, :, 384:512],
                                mw2[:, :, 0:128], OP.add)
                    e12 = pe_pool.tile([128, 2, 512], fp16, tag="e",
                                       name=f"e_{ki}_{h}_{c}")
                    if p == 3:
                        nc.gpsimd.memset(e12[:, :, 256:384], 0.0)
                        nc.scalar.activation(e12[:, :, 384:512],
                                             s12[:, :, 384:512], AF.Exp,
                                             scale=SCALE)
                    else:
                        nc.scalar.activation(e12[:, :, c0:], s12[:, :, c0:],
                                             AF.Exp, scale=SCALE)
                    return e12, c0

                es = {0: emit_scores(0)}
                if la > 1 and klim > 1:
                    es[1] = emit_scores(1)
                U1 = pap.tile([128, 512], f32, tag="u", bufs=2, name=f"U1_{h}_{c}")
                U2 = pap.tile([128, 512], f32, tag="u", bufs=2, name=f"U2_{h}_{c}")
                for ki in range(klim):
                    if ki + la < klim:
                        es[ki + la] = emit_scores(ki + la)
                    if fillers and fstate is not None and not nofill:
                        fstate[0] += 1
                        due = fstate[0] * fstate[2] // fstate[1]
                        if ki == 0:
                            due = max(due, fstate[3] + 4)
                        while fstate[3] < due and fillers:
                            fillers.pop(0)()
                            fstate[3] += 1
                    e12, c0 = es.pop(ki)
                    st, sp = (ki == 0), (ki == klim - 1)
                    nc.tensor.matmul(U1[:, c0:], vt_sb[:, ki, ts(h, 128)],
                                     e12[:, 0, c0:], start=st, stop=sp)
                    nc.tensor.matmul(U2[:, c0:], vt_sb[:, ki, ts(h, 128)],
                                     e12[:, 1, c0:], start=st, stop=sp)
                    # denominator partials on DVE (fp16, 2x mode)
                    a0 = 384 if ki - 4 * c == 3 else c0
                    if ki == 0:
                        nc.gpsimd.tensor_scalar_mul(es12[:], e12[:], 1.0)
                    else:
                        nc.vector.tensor_tensor(es12[:, :, a0:], es12[:, :, a0:],
                                                e12[:, :, a0:], OP.add)
                    if ki == 2 and mid is not None:
                        mid()
                D12 = pap.tile([128, 2, 512], f32, tag="s", name=f"D_{h}_{c}")
                nc.tensor.matmul(D12[:, 0, :], ones16[:], es12[:, 0, :],
                                 start=True, stop=True, skip_group_check=True)
                nc.tensor.matmul(D12[:, 1, :], ones16[:], es12[:, 1, :],
                                 start=True, stop=True, skip_group_check=True)
                # epilogue (DVE/ACT): pre = U1/D1 - lbda*U2/D2
                r12 = pt.tile([128, 2, 512], fp16, tag="r1")
                nc.vector.reciprocal(out=r12[:], in_=D12[:])
                t1 = pt.tile([128, 512], fp16, tag="t1")
                t2 = pt.tile([128, 512], fp16, tag="t2")
                pre = pt.tile([128, 512], fp16, tag="pre", bufs=4,
                              name=f"pre_{h}_{c}")
                nc.vector.tensor_tensor(t1[:], U1[:], r12[:, 0, :], OP.mult)
                nc.vector.scalar_tensor_tensor(t2[:], r12[:, 1, :], float(lbda),
                                               U2[:], OP.mult, OP.mult)
                nc.vector.tensor_tensor(pre[:], t1[:], t2[:], OP.subtract)
                sq = pt.tile([128, 512], fp16, tag="sq", bufs=4,
                             name=f"sq_{h}_{c}")
                nc.gpsimd.tensor_tensor(sq[:], pre[:], pre[:], OP.mult)
                return pre, sq

            def emit_stats1(h, c, pre, sq, pool, mtag, mbufs):
                """LN stats matmuls + variance (no table swap)."""
                mu = pool.tile([128, 512], f32, tag=mtag, bufs=mbufs,
                               name=f"mu_{h}_{c}")
                ex2 = pool.tile([128, 512], f32, tag=mtag, bufs=mbufs,
                                name=f"ex2_{h}_{c}")
                nc.tensor.matmul(mu[:], ones128[:], pre[:], start=True,
                                 stop=True, skip_group_check=True)
                nc.tensor.matmul(ex2[:], ones128[:], sq[:], start=True,
                                 stop=True, skip_group_check=True)
                muS = pt.tile([128, 512], fp16, tag="muS", bufs=4,
                              name=f"muS_{h}_{c}")
                nc.vector.tensor_scalar_mul(muS[:], mu[:], 1.0)
                musq = pt.tile([128, 512], fp16, tag="t1")
                nc.gpsimd.tensor_tensor(musq[:], muS[:], muS[:], OP.mult)
                var = pt.tile([128, 512], fp16, tag="var", bufs=4,
                              name=f"var_{h}_{c}")
                nc.vector.tensor_tensor(var[:], ex2[:], musq[:], OP.subtract)
                return muS, var

            def emit_stats2(c, ctx):
                """Batched sqrt + LN apply (one act-table swap per chunk).
                ctx: list of (h, (pre, muS, var))."""
                ctx2 = ctx
                sds, t3s = {}, {}
                for h, (pre, muS, var) in ctx2:
                    sd = pt.tile([128, 512], fp16, tag="sd", bufs=4,
                                 name=f"sd_{h}_{c}")
                    nc.scalar.activation(sd[:], var[:], AF.Sqrt, bias=eps_sb[:])
                    sds[h] = sd
                for h, (pre, muS, var) in ctx2:
                    rstd = pt.tile([128, 512], fp16, tag="r2")
                    nc.vector.reciprocal(out=rstd[:], in_=sds[h][:])
                    cen = pt.tile([128, 512], fp16, tag="t1")
                    nc.vector.tensor_tensor(cen[:], pre[:], muS[:], OP.subtract)
                    t3 = pt.tile([128, 512], fp16, tag="t3", bufs=4,
                                 name=f"t3_{h}_{c}")
                    nc.vector.tensor_tensor(t3[:], cen[:], rstd[:], OP.mult)
                    t3s[h] = t3
                for h, _ in ctx2:
                    nc.scalar.activation(attn_sbs[c][:, h, :], t3s[h][:],
                                         AF.Identity,
                                         bias=gb_sb[:, HPC + h:HPC + h + 1],
                                         scale=gb_sb[:, h:h + 1])
                # force the exp table back in while ACT is off the critical path
                nc.scalar.activation(dummy_sb[:], eps_sb[:], AF.Exp, scale=1.0)

            def make_wo_fillers(c, pwo, worder=tuple(range(HPC))):
                """Build filler closures: one W_o psum group (+copy/DMA) each."""
                att = attn_sbs.pop(c)
                fillers = []
                state = {}

                def group(qi, half, nj2, part):
                    def emit():
                        if nj2 == 0 and part == 0:
                            state[(qi, half)] = pst.tile([128, D // 2], fp16,
                                                         tag="st",
                                                         name=f"st{c}_{qi}_{half}")
                        nj = half * 2 + nj2
                        if part == 0:
                            state[(qi, nj, "po")] = pwo.tile(
                                [128, 512], f32, tag="wo",
                                name=f"po{c}_{qi}_{nj}")
                        po = state[(qi, nj, "po")]
                        hs = worder[:2] if part == 0 else worder[2:]
                        for hi, hh in enumerate(hs):
                            nc.tensor.matmul(po[:], att[:, hh, ts(qi, 128)],
                                             wo_sb[:, hh, ts(nj, 512)],
                                             start=(part == 0 and hi == 0),
                                             stop=(part == 1 and hi == 1))
                        if part == 1:
                            stage = state[(qi, half)]
                            nc.vector.tensor_scalar_mul(
                                stage[:, ts(nj2, 512)], po[:], 1.0)
                            if nj2 == 1:
                                if c == NCH - 1 and qi == 3:
                                    # program tail: split across both DMA
                                    # queues so the drain halves
                                    nc.sync.dma_start(
                                        out[ds(c * 512 + qi * 128, 128),
                                            ds(half * (D // 2), 512)],
                                        stage[:, 0:512])
                                    nc.gpsimd.dma_start(
                                        out[ds(c * 512 + qi * 128, 128),
                                            ds(half * (D // 2) + 512, 512)],
                                        stage[:, 512:1024])
                                else:
                                    nc.sync.dma_start(
                                        out[ds(c * 512 + qi * 128, 128),
                                            ts(half, D // 2)],
                                        stage[:])
                    return emit

                for qi in range(4):
                    for half in range(2):
                        for nj2 in range(2):
                            for part in range(2):
                                fillers.append(group(qi, half, nj2, part))
                return fillers

            # wo weights late on SP (needed from ~chunk1)
            wo_sb = pw.tile([128, HPC, D], fp16)

            # ------------------ main chunk loop ----------------------------
            load_x(0, split=True)
            nc.sync.dma_start(idn_sb[:], idn[:])
            load_cs(0)
            load_x(1)
            load_cs(1)
            nc.sync.dma_start(wo_sb[:], wo3[:])
            for c in range(NCH):
                attn_sbs[c] = pat.tile([128, HPC, 512], fp16, tag="at",
                                       name=f"attn{c}")
                with tc.tile_pool(name=f"ptr{c}", bufs=1, space="PSUM") as ptr, \
                     tc.tile_pool(name=f"pq{c}", bufs=2, space="PSUM") as pq, \
                     tc.tile_pool(name=f"pk{c}", bufs=1, space="PSUM") as pk, \
                     tc.tile_pool(name=f"pv{c}", bufs=1, space="PSUM") as pv, \
                     tc.tile_pool(name=f"pso{c}", bufs=1, space="PSUM") as pso:
                    emit_proj(c, pq, pk, pv, ptr, pso)
                if c + 2 < NCH:
                    load_x(c + 2)
                    load_cs(c + 2)
                horder = list(range(HPC)) if c < NCH - 1 else [3, 0, 1, 2]
                use_filler = (c > 0)
                sbufs = 2 if use_filler else 3
                with tc.tile_pool(name=f"pap{c}", bufs=sbufs,
                                  space="PSUM") as pap, \
                     tc.tile_pool(name=f"pwo{c}", bufs=2, space="PSUM") as pwo:
                    fillers = make_wo_fillers(c - 1, pwo) if use_filler else None
                    # fstate: [opportunities seen, total opportunities,
                    #          fillers total, fillers emitted]
                    fstate = [0, (4 * c + 4) * HPC, 32, 0]
                    ctx = []
                    prevs = None
                    for h in horder:
                        mid = None
                        if prevs is not None:
                            ph, pp, ss = prevs

                            def mid(hh=ph, pp=pp, ss=ss, cx=ctx):
                                muS, var = emit_stats1(hh, c, pp, ss, pap,
                                                       "u", 2)
                                cx.append((hh, (pp, muS, var)))
                        pre, sq = emit_attn_iter(h, c, pap, mid, fillers,
                                                 fstate,
                                                 la=1 if use_filler else 2,
                                                 nofill=(h == horder[-1]))
                        prevs = (h, pre, sq)
                    if fillers:
                        for f in fillers:
                            f()
                        fillers = None

                    if c < NCH - 1:
                        def tail(pool, cc=c, pv=prevs, cx=ctx):
                            muS, var = emit_stats1(pv[0], cc, pv[1], pv[2],
                                                   pool, "so", 1)
                            cx.append((pv[0], (pv[1], muS, var)))
                            emit_stats2(cc, cx)
                        stats_tail[0] = tail
                    else:
                        muS, var = emit_stats1(prevs[0], c, prevs[1], prevs[2],
                                               pap, "u", 2)
                        ctx.append((prevs[0], (prevs[1], muS, var)))
                        emit_stats2(c, ctx)
            with tc.tile_pool(name="pwoF", bufs=4, space="PSUM") as pwo:
                for f in make_wo_fillers(NCH - 1, pwo, worder=(3, 0, 1, 2)):
                    f()

    nc.finalize()
    return nc


def get_program(s=S, lbda=0.5):
    key = (s, float(lbda))
    if key not in _PROGRAM_CACHE:
        _PROGRAM_CACHE[key] = build_program(s, lbda)
    return _PROGRAM_CACHE[key]


def make_core_inputs(x, cos, sin, W_qkv, W_o, ln_gamma, ln_beta, core, s=S):
    """Host-side shard prep for one core."""
    b, hg = core // 4, core % 4
    heads = list(range(hg * HPC, (hg + 1) * HPC))

    # qk cols: [E(256) | O(256)] per section; E layout (h, m, j) -> col 2j
    def eo_cols(base):
        ev = [base + hh * DH + m * HD + 2 * j
              for hh in heads for m in range(2) for j in range(32)]
        od = [c + 1 for c in ev]
        return ev + od

    cols = eo_cols(0) + eo_cols(D)            # q section, k section
    wqk = np.ascontiguousarray(W_qkv[:, cols]).astype(np.float16)
    vcols = [2 * D + hh * DH + dd for hh in heads for dd in range(DH)]
    wv = np.ascontiguousarray(W_qkv[:, vcols]).astype(np.float16)
    worows = [hh * DH + dd for hh in heads for dd in range(DH)]
    wo = np.ascontiguousarray(W_o[worows, :]).astype(np.float16)

    xT = np.ascontiguousarray(x[b].T).astype(np.float16)

    # cos/sin tables: [s, 256] with col (h*64 + m*32 + j) -> table[:, j]
    csT = np.tile(cos, (1, 8)).astype(np.float16)       # [s, 256]
    snT = np.tile(sin, (1, 8)).astype(np.float16)

    gbm = np.zeros((128, 2 * HPC), dtype=np.float32)
    for j, hh in enumerate(heads):
        gbm[:, j] = ln_gamma[hh] * OUT_MULT
        gbm[:, HPC + j] = ln_beta[hh] * OUT_MULT

    tri = np.where(np.triu(np.ones((128, 128), dtype=bool)), 0.0, NEG)
    mwide = np.tile(tri, (1, 2)).astype(np.float32)

    return {
        "xT": xT, "wqk": wqk, "wv": wv, "wo": wo, "csT": csT, "snT": snT,
        "gb": gbm, "mw": mwide,
        "idn": np.eye(128, dtype=np.float16),
    }


def _mask_is_causal(mask, s=S):
    m = np.asarray(mask).reshape(s, s)
    tril = np.tril(np.ones((s, s), dtype=bool))
    if not np.array_equal(m == 0.0, tril):
        return False
    off = m[~tril]
    return off.size == 0 or (np.all(off <= -1.0e8) and np.all(np.isfinite(off)))


def _numpy_reference(x, mask, cos, sin, W_qkv, W_o, ln_gamma, ln_beta, lbda):
    """Exact-math fallback (used only if the mask is not the causal pattern)."""
    b, s, d = x.shape
    qkv = x @ W_qkv
    q, k, v = np.split(qkv, 3, axis=-1)
    q = q.reshape(b, s, H, DH).transpose(0, 2, 1, 3)
    k = k.reshape(b, s, H, DH).transpose(0, 2, 1, 3)
    v = v.reshape(b, s, H, DH).transpose(0, 2, 1, 3)

    def rope(t):
        tr = t.reshape(b, H, s, HD // 2, 2)
        x1, x2 = tr[..., 0], tr[..., 1]
        c = cos[None, None]
        sn_ = sin[None, None]
        o1 = x1 * c - x2 * sn_
        o2 = x1 * sn_ + x2 * c
        return np.stack([o1, o2], axis=-1).reshape(b, H, s, HD)

    q1, q2 = q[..., :HD], q[..., HD:]
    k1, k2 = k[..., :HD], k[..., HD:]
    q1, k1 = rope(q1), rope(k1)
    q2, k2 = rope(q2), rope(k2)

    def softm(z):
        z = z - z.max(-1, keepdims=True)
        e = np.exp(z)
        return e / e.sum(-1, keepdims=True)

    m = np.asarray(mask).reshape(1, 1, s, s)
    a1 = softm(np.einsum("bhqd,bhkd->bhqk", q1, k1) * SCALE + m)
    a2 = softm(np.einsum("bhqd,bhkd->bhqk", q2, k2) * SCALE + m)
    a = a1 - float(lbda) * a2
    o = np.einsum("bhqk,bhkd->bhqd", a, v)
    mu = o.mean(-1, keepdims=True)
    var = o.var(-1, keepdims=True)
    o = (o - mu) / np.sqrt(var + 1e-5)
    o = o * ln_gamma[None, :, None, :] + ln_beta[None, :, None, :]
    o = o * OUT_MULT
    o = o.transpose(0, 2, 1, 3).reshape(b, s, d)
    return (o @ W_o).astype(np.float32)


def kernel(x, mask, cos, sin, W_qkv, W_o, ln_gamma, ln_beta, lbda):
    global LAST_RESULTS, LAST_EXEC_NS
    x = np.asarray(x, dtype=np.float32)
    cos = np.asarray(cos, dtype=np.float32)
    sin = np.asarray(sin, dtype=np.float32)
    W_qkv = np.asarray(W_qkv, dtype=np.float32)
    W_o = np.asarray(W_o, dtype=np.float32)
    ln_gamma = np.asarray(ln_gamma, dtype=np.float32)
    ln_beta = np.asarray(ln_beta, dtype=np.float32)
    lbda_f = float(np.asarray(lbda))

    if not _mask_is_causal(mask):
        return _numpy_reference(x, mask, cos, sin, W_qkv, W_o,
                                ln_gamma, ln_beta, lbda_f)

    from concourse.bass_utils import run_bass_kernel_spmd

    nc = get_program(S, lbda_f)
    in_maps = [
        make_core_inputs(x, cos, sin, W_qkv, W_o, ln_gamma, ln_beta, c)
        for c in range(NCORES)
    ]
    kwargs = {"trace": TRACE}
    if TRACE and TRACE_DIR:
        kwargs["tmpdir"] = TRACE_DIR
    res = run_bass_kernel_spmd(nc, in_maps, core_ids=list(range(NCORES)),
                               **kwargs)
    LAST_RESULTS = res
    LAST_EXEC_NS = getattr(res, "exec_time_ns", None)

    outf = np.zeros((B, S, D), dtype=np.float32)
    for c in range(NCORES):
        outf[c // 4] += res.results[c]["out"].astype(np.float32)
    return outf
